# revision 26
# baseline (speedup 1.0000x reference)
"""Causal MHA attention-out kernel for TRN2, head-sharded across 8 NeuronCores.

Reference computation (fp32):
    scores = (q @ k^T) / sqrt(64), causal mask, softmax
    z      = pattern @ v
    out    = sum_h z_h @ W_O[h] + b_O          (residual passed through)

Sharding: 16 heads -> 8 cores x 2 adjacent heads. Each core computes a
partial out (its 2 heads' contribution, both batches); host sums partials.

Per-core layout (per batch b):
  kT/qT  [128, 2048]   d-major (head0 -> partitions 0-63, head1 -> 64-127),
                       loaded via bf16 xbar DMA transpose (dma_start_transpose).
  Pipeline unit = one k-block (128 k positions) for a 512-wide q chunk,
  BOTH heads sharing one [128,1024] fp32 PSUM score tile (h0 cols 0-511,
  h1 cols 512-1023). The ACT engine is the softmax-exp bottleneck (~78us
  of exp per core is irreducible: no other engine has activation tables),
  so ACT does exp ONLY -- one instruction per off-diagonal unit, two per
  diagonal unit. Causal masking is folded into the scores: a constant
  -16384 strict-upper-triangle is matmul'd (ident @ triu) into each
  diagonal 128x128 score block before QK accumulates onto it, so exp
  yields exact zeros there and PV depends on nothing but exp. Normalize
  runs on DVE (reciprocal + mul) with a Pool partition_broadcast.
  Projection goes PSUM -> bf16 osb staging (copies split 50/50 between
  DVE and ACT: HW calibration measured DVE work carrying the highest
  critical-path coefficient while ACT had slack) -> one consolidated
  SWDGE out-DMA per (b,qc) issued from Pool.

  Engine-queue roles: SP carries ONLY input loads (4 whole-tensor xbar
  transposes + v loads), so its program runs a full For_i iteration ahead
  of compute -- true cross-iteration prefetch; Pool owns out-DMAs.

  Emission is one globally software-pipelined stream over all (b,qc,kb)
  units: QK(u) leads PV(u) by PV_LAG units so the in-order PE sequencer
  never blocks on the exp chain (PE p-state: a continuously-busy PE runs
  2x faster than one that keeps stalling); chunk tails never stall the
  next chunk's QKs. Projection ops are deferred and injected one per unit,
  no earlier than INJECT_DELAY units after their normalize, so a
  not-yet-ready zsb never head-of-line-blocks the PE sequencer.
  PSUM: 3x2-bank score slots (shared with projection outputs) + 2x1-bank
  z accumulators = 8 banks exactly.
"""

import numpy as np

import concourse.bass as bass
import concourse.mybir as mybir
from concourse import bacc
import concourse.tile as tile
from concourse.bass_utils import run_bass_kernel_spmd

B = 2
S = 2048
D_MODEL = 1024
N_HEADS = 16
D_HEAD = 64
N_CORES = 8
HPC = 2  # heads per core
CW = HPC * D_HEAD  # 128 columns of q/k/v per core
NKB = S // 128  # 16 k-blocks
NQC = S // 512  # 4 q-chunks
INV_SCALE = 1.0 / 8.0  # 1/sqrt(64)

F32 = mybir.dt.float32
MMDT = mybir.dt.bfloat16  # matmul operand dtype: guaranteed 1 cyc/row on PE

import os
PV_LAG = int(os.environ.get("PV_LAG", "4"))  # units QK leads PV
INJECT_DELAY = int(os.environ.get("INJECT_DELAY", "3"))  # units QK runs ahead of PV

_CACHE = {}


def _build_bass(reps=None, py_reps=1):
    nc = bacc.Bacc("TRN2", target_bir_lowering=False)

    # per-core: ONE batch, FOUR heads (2 head-groups of 2). q/k/v columns are
    # the core's 4 heads; wo is the core's [4*64, D_MODEL] slice; out is the
    # core's partial for its batch (host sums 4 cores per batch).
    q_d = nc.dram_tensor("q", [S, 2 * CW], MMDT, kind="ExternalInput")
    k_d = nc.dram_tensor("k", [S, 2 * CW], MMDT, kind="ExternalInput")
    v_d = nc.dram_tensor("v", [S, 2 * CW], MMDT, kind="ExternalInput")
    wo_d = nc.dram_tensor("wo", [2 * CW, D_MODEL], MMDT, kind="ExternalInput")
    out_d = nc.dram_tensor("out", [S, D_MODEL], MMDT, kind="ExternalOutput")

    with tile.TileContext(nc) as tc:
        with (
            tc.tile_pool(name="const", bufs=1) as const_pool,
            tc.tile_pool(name="big", bufs=4) as big_pool,
            tc.tile_pool(name="stage", bufs=4) as stage_pool,
            tc.tile_pool(name="pat", bufs=8) as pat_pool,
            tc.tile_pool(name="osb", bufs=4) as osb_pool,
            tc.tile_pool(name="psc", bufs=3, space="PSUM") as psc_pool,
            tc.tile_pool(name="pz", bufs=2, space="PSUM") as pz_pool,
        ):
            ones16 = const_pool.tile([128, NKB], F32)
            nc.gpsimd.memset(ones16, 1.0)
            from concourse.masks import make_identity
            ident_f = const_pool.tile([128, 128], F32)
            make_identity(nc, ident_f)
            ident = const_pool.tile([128, 128], MMDT)
            nc.vector.tensor_copy(ident, ident_f)
            # triu_neg[p, j] = -16384 where p > j else 0 (strict upper tri in
            # [k, q] coords = the causally-masked half of a diagonal block)
            triu_f = const_pool.tile([128, 128], F32)
            nc.gpsimd.memset(triu_f, -16384.0)
            nc.gpsimd.affine_select(
                out=triu_f,
                in_=triu_f,
                compare_op=mybir.AluOpType.is_ge,
                fill=0.0,
                base=-1,
                pattern=[[-1, 128]],
                channel_multiplier=1,
            )
            triu_neg = const_pool.tile([128, 128], MMDT)
            nc.vector.tensor_copy(triu_neg, triu_f)
            wo_sbs = []
            for hg in range(2):
                wo_sb = const_pool.tile([CW, D_MODEL], MMDT, name=f"wo{hg}")
                nc.sync.dma_start(wo_sb, wo_d[hg * CW : (hg + 1) * CW, :])
                wo_sbs.append(wo_sb)

            import contextlib

            # UNROLL>1 emits several bodies per hardware-loop trip (step =
            # UNROLL keeps total body count == reps): inside For_i the tile
            # rings use FIXED addresses per trip, so cross-iteration
            # double-buffering (input DMA of body n+1 overlapping body n's
            # compute) only happens between the unrolled bodies of one trip.
            unroll = int(os.environ.get("UNROLL", "2")) if reps else 1
            loop_cm = (
                tc.For_i(
                    0,
                    reps,
                    unroll,
                    hint_engines=(
                        mybir.EngineType.PE,
                        mybir.EngineType.DVE,
                        mybir.EngineType.Activation,
                        mybir.EngineType.Pool,
                        mybir.EngineType.SP,
                    ),
                    staggered_reset=True,
                )
                if reps
                else contextlib.nullcontext()
            )
            with loop_cm:
                for _pr in range(py_reps * unroll):
                    _emit_body(nc, tc, locals())
    nc.compile()
    return nc


def _emit_body(nc, tc, env):
    (q_d, k_d, v_d, wo_d, out_d) = (
        env["q_d"], env["k_d"], env["v_d"], env["wo_d"], env["out_d"]
    )
    (const_pool, big_pool, stage_pool, pat_pool, psc_pool, pz_pool) = (
        env["const_pool"], env["big_pool"], env["stage_pool"], env["pat_pool"],
        env["psc_pool"], env["pz_pool"]
    )
    osb_pool = env["osb_pool"]
    ones16, wo_sbs = env["ones16"], env["wo_sbs"]
    ident, triu_neg = env["ident"], env["triu_neg"]
    # calibration knobs: duplicate one engine's instructions to measure the
    # HW marginal cost of that engine (timing builds only)
    dup_exp = int(os.environ.get("DUP_EXP", "1"))
    dup_mm = int(os.environ.get("DUP_MM", "1"))
    dup_dve = int(os.environ.get("DUP_DVE", "1"))
    dup_tp = int(os.environ.get("DUP_TPOSE", "1"))

    kTs, qTs, vbigs = [], [], []
    for hg in range(2):
        cols = slice(hg * CW, (hg + 1) * CW)
        kT = big_pool.tile([128, S], MMDT, tag="kT", name=f"kT{hg}")
        qT = big_pool.tile([128, S], MMDT, tag="qT", name=f"qT{hg}")
        # v packed per k-block as [v_h0 | ones*64 | v_h1 | ones*64] (256 cols):
        # the 64 replicated ones columns make PV emit the softmax denominator
        # replicated across partitions 64-127, so normalize needs NO Pool
        # partition_broadcast -- reciprocal runs on [64,512] directly and
        # tensor_mul aligns partition-wise. PV stream cost is unchanged
        # (M 65->128 is the out-partition dim, not the streamed dim).
        vbig = big_pool.tile([128, NKB * 256], MMDT, tag="vb", name=f"vb{hg}")
        kTs.append(kT); qTs.append(qT); vbigs.append(vbig)
        # bf16 enables the xbar DMA transpose: one transposing DMA
        # per tensor replaces PE transposes + DVE copies entirely.
        # SP carries ONLY input loads: issuing a transpose costs ~0.7us of
        # sequencer time, and with nothing queued behind them SP's program
        # runs a full For_i iteration ahead -- true cross-iteration prefetch.
        # Out-DMAs go to Pool's SWDGE queue instead.
        for src_, dstT in ((k_d, kT), (q_d, qT)):
            for _d in range(dup_tp):
                nc.sync.dma_start_transpose(dstT, src_[:, cols])
        v3 = vbig.rearrange("p (t c) -> p t c", c=256)
        vsrc = v_d.rearrange("(t p) c -> p t c", p=128)
        nc.sync.dma_start(v3[:, :, 0:64], vsrc[:, :, hg * CW : hg * CW + 64])
        nc.sync.dma_start(
            v3[:, :, 128:192], vsrc[:, :, hg * CW + 64 : hg * CW + 128]
        )
        # all ones blocks in one 3D-AP memset: [128, 2*NKB, 64]
        v4 = vbig.rearrange("p (t c) -> p t c", c=128)
        nc.vector.memset(v4[:, :, 64:128], 1.0)

    # Deferred projection ops of completed (b,qc) chunks: injected between
    # units of later chunks so the PE/psc ring never drains.
    pending = []  # list of (earliest_index, closure) emitting one proj op
    cur_idx = [0]

    def emit_some_pending(k):
        n = 0
        while pending and n < k and pending[0][0] <= cur_idx[0]:
            pending.pop(0)[1]()
            n += 1

    def make_proj(qc, zsb0, zsb1, tail=False):
        # one [128, 4*1024] staging tile per qc; both head-groups accumulate
        # into the same PSUM op tile, then a single consolidated out-DMA on
        # Pool's SWDGE queue (994ns fixed cost per DMA, so batch)
        osb = osb_pool.tile([128, 4 * D_MODEL], MMDT, tag="osb",
                            name=f"osb{qc}")

        def one_op(qb):
            def emit():
                op = psc_pool.tile([128, 1024], F32, tag="sc",
                                   name=f"op{qc}_{qb}")
                for mch in range(2):
                    for hg, zsb in ((0, zsb0), (1, zsb1)):
                        nc.tensor.matmul(
                            op[:, mch * 512 : (mch + 1) * 512],
                            lhsT=zsb[:, qb * 128 : (qb + 1) * 128],
                            rhs=wo_sbs[hg][:, mch * 512 : (mch + 1) * 512],
                            start=(hg == 0),
                            stop=(hg == 1),
                        )
                dst = osb[:, qb * 1024 : (qb + 1) * 1024]
                copy_eng = os.environ.get("COPY_ENG", "dve")
                for _d in range(dup_dve):
                    # engine choice for the PSUM->SBUF staging copy
                    if copy_eng == "act" or (copy_eng == "split" and qb % 2 == 1):
                        nc.scalar.copy(dst, op)
                    elif copy_eng == "pool" or (copy_eng == "dvepool" and qb % 2 == 1):
                        nc.gpsimd.tensor_copy(dst, op)
                    else:
                        nc.vector.tensor_copy(dst, op)
                if qb == 3:
                    ddst = out_d[qc * 512 : (qc + 1) * 512, :].rearrange(
                        "(qb p) m -> p qb m", p=128
                    )
                    src = osb.rearrange("p (qb m) -> p qb m", m=D_MODEL)
                    nc.gpsimd.dma_start(ddst, src)
            return emit

        return [one_op(qb) for qb in range(4)]

    # One globally software-pipelined stream over every (hg, qc, kb) unit:
    # head-groups interleave at chunk granularity, QK leads PV by PV_LAG
    # units, and chunk tails (PV drain / normalize / projection) never stall
    # the in-order PE sequencer because the next chunk's QKs are emitted first.
    stream = []
    for qc in range(NQC):
        for b in range(B):
            for kb in range(4 * qc + 4):
                stream.append((b, qc, kb))

    zaccs = {}  # (hg, qc) -> [h0, h1] PSUM accumulators
    zsbs = {}   # (hg, qc) -> normalized z staging tile
    pats = {}   # (hg, qc, kb) -> pattern tile

    def emit_qk_exp(u):
        b, qc, kb = u
        kT, qT = kTs[b], qTs[b]
        dd = kb - 4 * qc
        s = 128 * dd if dd > 0 else 0
        sc = psc_pool.tile([128, 1024], F32, tag="sc", name=f"sc{b}_{qc}_{kb}")
        for h in range(HPC):
            # one matmul per head: diagonal blocks are exp'd unmasked and the
            # causally-masked triangle is zeroed on the PATTERN afterwards by
            # a Pool affine_select (saves a triu matmul + a split QK matmul)
            for _d in range(dup_mm):
                nc.tensor.matmul(
                    sc[:, 512 * h + s : 512 * h + 512],
                    lhsT=kT[64 * h : 64 * h + 64, kb * 128 : (kb + 1) * 128],
                    rhs=qT[64 * h : 64 * h + 64, qc * 512 + s : (qc + 1) * 512],
                    start=True,
                    stop=True,
                )
        pt = pat_pool.tile([128, 1024], MMDT, tag="pat", name=f"pat{b}_{qc}_{kb}")
        pats[u] = pt
        # exp (ACT reads PSUM, scale=1/8 folded in); one instruction for
        # off-diagonal units, two for diagonal (skip the masked-out cols)
        if dd <= 0:
            eranges = [(0, 1024)]
        elif os.environ.get("EXP3D", "1") == "1":
            # single 3D-AP call covering both heads' unmasked columns
            pt3 = pt.rearrange("p (h q) -> p h q", q=512)
            sc3 = sc.rearrange("p (h q) -> p h q", q=512)
            for _d in range(dup_exp):
                nc.scalar.activation(
                    pt3[:, :, s:512],
                    sc3[:, :, s:512],
                    mybir.ActivationFunctionType.Exp,
                    scale=INV_SCALE,
                )
            eranges = []
        else:
            eranges = [(s, 512), (512 + s, 1024)]
        for e0, e1 in eranges:
            for _d in range(dup_exp):
                nc.scalar.activation(
                    pt[:, e0:e1],
                    sc[:, e0:e1],
                    mybir.ActivationFunctionType.Exp,
                    scale=INV_SCALE,
                )
        if dd >= 0:
            # zero the causally-masked triangle (k > q) of both heads' diag
            # blocks on the pattern: keep where j - p >= 0, else fill 0.
            # [128, 2, 128] 3D AP covers both heads in one Pool op.
            pt3 = pt.rearrange("p (h q) -> p h q", q=512)
            nc.gpsimd.affine_select(
                out=pt3[:, :, s : s + 128],
                in_=pt3[:, :, s : s + 128],
                compare_op=mybir.AluOpType.is_ge,
                fill=0.0,
                base=0,
                pattern=[[0, 2], [1, 128]],
                channel_multiplier=-1,
            )

    def emit_pv(u):
        b, qc, kb = u
        last_kb = 4 * qc + 3
        if kb == 0:
            zaccs[(b, qc)] = [
                pz_pool.tile([128, 512], F32, tag="z", name=f"zacc{b}_{qc}_{h}")
                for h in range(HPC)
            ]
        zacc = zaccs[(b, qc)]
        dd = kb - 4 * qc
        s = 128 * dd if dd > 0 else 0
        for h in range(HPC):
            for _d in range(dup_mm):
                nc.tensor.matmul(
                    zacc[h][:, s:512],
                    lhsT=vbigs[b][:, kb * 256 + 128 * h : kb * 256 + 128 * h + 128],
                    rhs=pats[u][:, 512 * h + s : 512 * h + 512],
                    start=(kb == 0 and _d == 0),
                    stop=(kb == last_kb and _d == dup_mm - 1),
                )
        if kb == last_kb:
            emit_normalize(b, qc)

    def emit_normalize(hg, qc):
        # normalize: z = z / denom  (DVE reciprocal + mul, Pool broadcast)
        zacc = zaccs[(hg, qc)]
        zsb = stage_pool.tile([128, 512], MMDT, tag="zsb", name=f"zsb{hg}_{qc}")
        zsbs[(hg, qc)] = zsb
        # denominator rows 64-127 are already replicated (ones*64 in PV
        # weights): reciprocal on [64,512] costs the same as [1,512] on DVE
        # (free-dim serial, partitions parallel) and tensor_mul aligns
        # partition-wise with zacc rows 0-63 -- no Pool broadcast hop.
        rbs = []
        for h in range(HPC):
            r_sb = stage_pool.tile([64, 512], F32, tag="r")
            nc.vector.reciprocal(r_sb, zacc[h][64:128, :])
            rbs.append(r_sb)
        for h in range(HPC):
            for _d in range(dup_dve):
                nc.vector.tensor_mul(
                    zsb[64 * h : 64 * h + 64, :],
                    zacc[h][0:64, :],
                    rbs[h],
                )
        if hg == 1:
            ops = make_proj(qc, zsbs[(0, qc)], zsbs[(1, qc)],
                            tail=(qc == NQC - 1
                                  and os.environ.get("TAIL_ACT", "1") == "1"))
            pending.extend((cur_idx[0] + INJECT_DELAY, op) for op in ops)

    for i, u in enumerate(stream):
        cur_idx[0] = i
        emit_qk_exp(u)
        emit_some_pending(1)
        if i >= PV_LAG:
            emit_pv(stream[i - PV_LAG])
    for u in stream[-PV_LAG:]:
        emit_some_pending(1)
        emit_pv(u)

    # drain whatever projection ops remain at the end of the iteration
    cur_idx[0] = float("inf")
    emit_some_pending(len(pending))


def make_in_maps(q, k, v, W_O):
    import ml_dtypes

    bf16 = ml_dtypes.bfloat16
    q = np.asarray(q, dtype=np.float32).astype(bf16)
    k = np.asarray(k, dtype=np.float32).astype(bf16)
    v = np.asarray(v, dtype=np.float32).astype(bf16)
    W_O = np.asarray(W_O, dtype=np.float32).astype(bf16)
    in_maps = []
    for c in range(N_CORES):
        b = c // 4  # batch owned by this core
        g = c % 4   # head-group of 4 heads
        cols = slice(g * 2 * CW, (g + 1) * 2 * CW)
        in_maps.append(
            {
                "q": np.ascontiguousarray(q[b, :, cols]),
                "k": np.ascontiguousarray(k[b, :, cols]),
                "v": np.ascontiguousarray(v[b, :, cols]),
                "wo": np.ascontiguousarray(
                    W_O[g * 4 : (g + 1) * 4].reshape(2 * CW, D_MODEL)
                ),
            }
        )
    return in_maps


def get_nc():
    if "nc" not in _CACHE:
        _CACHE["nc"] = _build_bass()
    return _CACHE["nc"]


def kernel(q, k, v, residual, W_O, b_O):
    nc = get_nc()
    in_maps = make_in_maps(q, k, v, W_O)
    res = run_bass_kernel_spmd(nc, in_maps, core_ids=list(range(N_CORES)))
    out = np.zeros((B, S, D_MODEL), dtype=np.float64)
    for c, r in enumerate(res.results):
        out[c // 4] += r["out"].astype(np.float64)
    out = (out + np.asarray(b_O, dtype=np.float64)[None, None, :]).astype(np.float32)
    return out, np.asarray(residual)



# revision 31
# speedup vs baseline: 1.1765x; 1.1765x over previous
"""Causal MHA attention-out kernel for TRN2, sharded (batch x head-group)
across 8 NeuronCores.

Reference computation (fp32):
    scores = (q @ k^T) / sqrt(64), causal mask, softmax
    z      = pattern @ v
    out    = sum_h z_h @ W_O[h] + b_O          (residual passed through)

Sharding: core c owns batch c//4 and heads 4*(c%4)..4*(c%4)+4 (2
head-groups of 2 heads). Each core computes a full [S, D_MODEL] partial
for its batch (both head-groups accumulate in PSUM before the output
projection is staged/written, halving staging copies and out-DMA bytes
vs head-only sharding); host sums 4 partials per batch.

Per-core layout (per head-group hg):
  kT/qT  [128, 2048]   d-major (head0 -> partitions 0-63, head1 -> 64-127),
                       loaded via bf16 xbar DMA transpose (dma_start_transpose).
  Pipeline unit = one k-block (128 k positions) for a 512-wide q chunk,
  BOTH heads sharing one [128,1024] fp32 PSUM score tile (h0 cols 0-511,
  h1 cols 512-1023). ACT does exp ONLY -- one instruction per unit (the
  diagonal's two disjoint unmasked ranges go through a single 3D-AP call).
  Causal masking is folded into the scores: a constant -16384
  strict-upper-triangle is matmul'd (ident @ triu) into each diagonal
  128x128 score block before QK accumulates onto it, so exp yields exact
  zeros there and PV depends on nothing but exp. (Masking the PATTERN
  with a Pool affine_select instead measured +29us on HW: an extra
  cross-engine hop on the exp->PV chain costs far more than 8.2k saved
  PE stream columns.)
  v is packed per k-block as [v_h0 | ones*64 | v_h1 | ones*64] so PV
  emits the softmax denominator REPLICATED on partitions 64-127: the
  normalize is then DVE-only (reciprocal on [64,512] + tensor_mul, no
  Pool partition_broadcast hop; reciprocal on [64,512] costs the same as
  on [1,512] since DVE time is free-dim-serial, partition-parallel).
  Projection (injected, deferred): 4 matmuls per op tile (2 head-groups x
  2 D_MODEL halves) -> bf16 osb staging copy -> one consolidated SWDGE
  out-DMA per q-chunk issued from Pool (994ns fixed cost per DMA).

  Engine-queue roles: SP carries ONLY input loads (4 whole-tensor xbar
  transposes + v loads); Pool owns out-DMAs.

  Emission is one globally software-pipelined stream over all (hg,qc,kb)
  units: QK(u) leads PV(u) by PV_LAG=4 units (PV_LAG=6 measured +45us on
  HW -- pat/queue pressure; do NOT deepen). Projection ops are deferred
  and injected one per unit, no earlier than INJECT_DELAY units after
  their normalize. PSUM: 3x2-bank score slots (shared with projection
  outputs) + 2x1-bank z accumulators = 8 banks exactly.

  The timed For_i path unrolls TWO bodies per hardware-loop trip
  (step=UNROLL): inside For_i the tile rings use fixed addresses per
  trip, so cross-iteration double-buffering only happens between the
  unrolled bodies (measured ~10us/iter on HW vs UNROLL=1).

HW A/B results (median of paired loop-differencing, this container):
  head-only sharding baseline 181us -> hg-sharding 172 -> +EXP3D/wide-ones
  171.5 -> +UNROLL=2 168. Marginal-cost calibration (DUP_* knobs):
  PE +0.91/us-busy, DVE +0.79, ACT +0.58 -> PE stream time is the most
  critical resource; cross-engine chain latency is the next.
"""

import numpy as np

import concourse.bass as bass
import concourse.mybir as mybir
from concourse import bacc
import concourse.tile as tile
from concourse.bass_utils import run_bass_kernel_spmd

B = 2
S = 2048
D_MODEL = 1024
N_HEADS = 16
D_HEAD = 64
N_CORES = 8
HPC = 2  # heads per core
CW = HPC * D_HEAD  # 128 columns of q/k/v per core
NKB = S // 128  # 16 k-blocks
NQC = S // 512  # 4 q-chunks
INV_SCALE = 1.0 / 8.0  # 1/sqrt(64)

F32 = mybir.dt.float32
MMDT = mybir.dt.bfloat16  # matmul operand dtype: guaranteed 1 cyc/row on PE

import os
PV_LAG = int(os.environ.get("PV_LAG", "4"))  # units QK leads PV
INJECT_DELAY = int(os.environ.get("INJECT_DELAY", "3"))  # units QK runs ahead of PV

_CACHE = {}


def _build_bass(reps=None, py_reps=1):
    nc = bacc.Bacc("TRN2", target_bir_lowering=False)

    # per-core: ONE batch, FOUR heads (2 head-groups of 2). q/k/v columns are
    # the core's 4 heads; wo is the core's [4*64, D_MODEL] slice; out is the
    # core's partial for its batch (host sums 4 cores per batch).
    q_d = nc.dram_tensor("q", [S, 2 * CW], MMDT, kind="ExternalInput")
    k_d = nc.dram_tensor("k", [S, 2 * CW], MMDT, kind="ExternalInput")
    v_d = nc.dram_tensor("v", [S, 2 * CW], MMDT, kind="ExternalInput")
    wo_d = nc.dram_tensor("wo", [2 * CW, D_MODEL], MMDT, kind="ExternalInput")
    out_d = nc.dram_tensor("out", [S, D_MODEL], MMDT, kind="ExternalOutput")

    with tile.TileContext(nc) as tc:
        with (
            tc.tile_pool(name="const", bufs=1) as const_pool,
            tc.tile_pool(name="big", bufs=4) as big_pool,
            tc.tile_pool(name="stage", bufs=4) as stage_pool,
            tc.tile_pool(name="pat", bufs=8) as pat_pool,
            tc.tile_pool(name="osb", bufs=4) as osb_pool,
            tc.tile_pool(name="psc", bufs=3, space="PSUM") as psc_pool,
            tc.tile_pool(name="pz", bufs=2, space="PSUM") as pz_pool,
        ):
            from concourse.masks import make_identity
            ident_f = const_pool.tile([128, 128], F32)
            make_identity(nc, ident_f)
            ident = const_pool.tile([128, 128], MMDT)
            nc.vector.tensor_copy(ident, ident_f)
            # triu_neg[p, j] = -16384 where p > j else 0 (strict upper tri in
            # [k, q] coords = the causally-masked half of a diagonal block)
            triu_f = const_pool.tile([128, 128], F32)
            nc.gpsimd.memset(triu_f, -16384.0)
            nc.gpsimd.affine_select(
                out=triu_f,
                in_=triu_f,
                compare_op=mybir.AluOpType.is_ge,
                fill=0.0,
                base=-1,
                pattern=[[-1, 128]],
                channel_multiplier=1,
            )
            triu_neg = const_pool.tile([128, 128], MMDT)
            nc.vector.tensor_copy(triu_neg, triu_f)
            wo_sbs = []
            for hg in range(2):
                wo_sb = const_pool.tile([CW, D_MODEL], MMDT, name=f"wo{hg}")
                nc.sync.dma_start(wo_sb, wo_d[hg * CW : (hg + 1) * CW, :])
                wo_sbs.append(wo_sb)

            import contextlib

            # UNROLL>1 emits several bodies per hardware-loop trip (step =
            # UNROLL keeps total body count == reps): inside For_i the tile
            # rings use FIXED addresses per trip, so cross-iteration
            # double-buffering (input DMA of body n+1 overlapping body n's
            # compute) only happens between the unrolled bodies of one trip.
            unroll = int(os.environ.get("UNROLL", "2")) if reps else 1
            loop_cm = (
                tc.For_i(
                    0,
                    reps,
                    unroll,
                    hint_engines=(
                        mybir.EngineType.PE,
                        mybir.EngineType.DVE,
                        mybir.EngineType.Activation,
                        mybir.EngineType.Pool,
                        mybir.EngineType.SP,
                    ),
                    staggered_reset=True,
                )
                if reps
                else contextlib.nullcontext()
            )
            with loop_cm:
                for _pr in range(py_reps * unroll):
                    _emit_body(nc, tc, locals())
    nc.compile()
    return nc


def _emit_body(nc, tc, env):
    (q_d, k_d, v_d, wo_d, out_d) = (
        env["q_d"], env["k_d"], env["v_d"], env["wo_d"], env["out_d"]
    )
    (const_pool, big_pool, stage_pool, pat_pool, psc_pool, pz_pool) = (
        env["const_pool"], env["big_pool"], env["stage_pool"], env["pat_pool"],
        env["psc_pool"], env["pz_pool"]
    )
    osb_pool = env["osb_pool"]
    wo_sbs = env["wo_sbs"]
    ident, triu_neg = env["ident"], env["triu_neg"]
    # calibration knobs: duplicate one engine's instructions to measure the
    # HW marginal cost of that engine (timing builds only)
    dup_exp = int(os.environ.get("DUP_EXP", "1"))
    dup_mm = int(os.environ.get("DUP_MM", "1"))
    dup_dve = int(os.environ.get("DUP_DVE", "1"))
    dup_tp = int(os.environ.get("DUP_TPOSE", "1"))

    kTs, qTs, vbigs = [], [], []
    for hg in range(2):
        cols = slice(hg * CW, (hg + 1) * CW)
        kT = big_pool.tile([128, S], MMDT, tag="kT", name=f"kT{hg}")
        qT = big_pool.tile([128, S], MMDT, tag="qT", name=f"qT{hg}")
        # v packed per k-block as [v_h0 | ones*64 | v_h1 | ones*64] (256 cols):
        # the 64 replicated ones columns make PV emit the softmax denominator
        # replicated across partitions 64-127, so normalize needs NO Pool
        # partition_broadcast -- reciprocal runs on [64,512] directly and
        # tensor_mul aligns partition-wise. PV stream cost is unchanged
        # (M 65->128 is the out-partition dim, not the streamed dim).
        vbig = big_pool.tile([128, NKB * 256], MMDT, tag="vb", name=f"vb{hg}")
        kTs.append(kT); qTs.append(qT); vbigs.append(vbig)
        # bf16 enables the xbar DMA transpose: one transposing DMA
        # per tensor replaces PE transposes + DVE copies entirely.
        # SP carries ONLY input loads: issuing a transpose costs ~0.7us of
        # sequencer time, and with nothing queued behind them SP's program
        # runs a full For_i iteration ahead -- true cross-iteration prefetch.
        # Out-DMAs go to Pool's SWDGE queue instead.
        for src_, dstT in ((k_d, kT), (q_d, qT)):
            for _d in range(dup_tp):
                nc.sync.dma_start_transpose(dstT, src_[:, cols])
        v3 = vbig.rearrange("p (t c) -> p t c", c=256)
        vsrc = v_d.rearrange("(t p) c -> p t c", p=128)
        nc.sync.dma_start(v3[:, :, 0:64], vsrc[:, :, hg * CW : hg * CW + 64])
        nc.sync.dma_start(
            v3[:, :, 128:192], vsrc[:, :, hg * CW + 64 : hg * CW + 128]
        )
        # all ones blocks in one 3D-AP memset: [128, 2*NKB, 64]
        v4 = vbig.rearrange("p (t c) -> p t c", c=128)
        nc.vector.memset(v4[:, :, 64:128], 1.0)

    # Deferred projection ops of completed (b,qc) chunks: injected between
    # units of later chunks so the PE/psc ring never drains.
    pending = []  # list of (earliest_index, closure) emitting one proj op
    cur_idx = [0]

    def emit_some_pending(k):
        n = 0
        while pending and n < k and pending[0][0] <= cur_idx[0]:
            pending.pop(0)[1]()
            n += 1

    def make_proj(qc, zsb0, zsb1, tail=False):
        # one [128, 4*1024] staging tile per qc; both head-groups accumulate
        # into the same PSUM op tile, then a single consolidated out-DMA on
        # Pool's SWDGE queue (994ns fixed cost per DMA, so batch)
        osb = osb_pool.tile([128, 4 * D_MODEL], MMDT, tag="osb",
                            name=f"osb{qc}")

        def one_op(qb):
            def emit():
                op = psc_pool.tile([128, 1024], F32, tag="sc",
                                   name=f"op{qc}_{qb}")
                for mch in range(2):
                    for hg, zsb in ((0, zsb0), (1, zsb1)):
                        nc.tensor.matmul(
                            op[:, mch * 512 : (mch + 1) * 512],
                            lhsT=zsb[:, qb * 128 : (qb + 1) * 128],
                            rhs=wo_sbs[hg][:, mch * 512 : (mch + 1) * 512],
                            start=(hg == 0),
                            stop=(hg == 1),
                        )
                dst = osb[:, qb * 1024 : (qb + 1) * 1024]
                copy_eng = os.environ.get("COPY_ENG", "dve")
                for _d in range(dup_dve):
                    # engine choice for the PSUM->SBUF staging copy
                    if copy_eng == "act" or (copy_eng == "split" and qb % 2 == 1):
                        nc.scalar.copy(dst, op)
                    elif copy_eng == "pool" or (copy_eng == "dvepool" and qb % 2 == 1):
                        nc.gpsimd.tensor_copy(dst, op)
                    else:
                        nc.vector.tensor_copy(dst, op)
                if qb == 3:
                    ddst = out_d[qc * 512 : (qc + 1) * 512, :].rearrange(
                        "(qb p) m -> p qb m", p=128
                    )
                    src = osb.rearrange("p (qb m) -> p qb m", m=D_MODEL)
                    nc.gpsimd.dma_start(ddst, src)
            return emit

        return [one_op(qb) for qb in range(4)]

    # One globally software-pipelined stream over every (hg, qc, kb) unit:
    # head-groups interleave at chunk granularity, QK leads PV by PV_LAG
    # units, and chunk tails (PV drain / normalize / projection) never stall
    # the in-order PE sequencer because the next chunk's QKs are emitted first.
    stream = []
    for qc in range(NQC):
        for b in range(B):
            for kb in range(4 * qc + 4):
                stream.append((b, qc, kb))

    zaccs = {}  # (hg, qc) -> [h0, h1] PSUM accumulators
    zsbs = {}   # (hg, qc) -> normalized z staging tile
    pats = {}   # (hg, qc, kb) -> pattern tile

    def emit_qk_exp(u):
        b, qc, kb = u
        kT, qT = kTs[b], qTs[b]
        dd = kb - 4 * qc
        s = 128 * dd if dd > 0 else 0
        sc = psc_pool.tile([128, 1024], F32, tag="sc", name=f"sc{b}_{qc}_{kb}")
        for h in range(HPC):
            if dd >= 0:
                nc.tensor.matmul(
                    sc[:, 512 * h + s : 512 * h + s + 128],
                    lhsT=ident,
                    rhs=triu_neg,
                    start=True,
                    stop=False,
                )
                for _d in range(dup_mm):
                    nc.tensor.matmul(
                        sc[:, 512 * h + s : 512 * h + s + 128],
                        lhsT=kT[64 * h : 64 * h + 64, kb * 128 : (kb + 1) * 128],
                        rhs=qT[64 * h : 64 * h + 64,
                               qc * 512 + s : qc * 512 + s + 128],
                        start=False,
                        stop=True,
                    )
                if s + 128 < 512:
                    for _d in range(dup_mm):
                        nc.tensor.matmul(
                            sc[:, 512 * h + s + 128 : 512 * h + 512],
                            lhsT=kT[64 * h : 64 * h + 64, kb * 128 : (kb + 1) * 128],
                            rhs=qT[64 * h : 64 * h + 64,
                                   qc * 512 + s + 128 : (qc + 1) * 512],
                            start=True,
                            stop=True,
                        )
            else:
                for _d in range(dup_mm):
                    nc.tensor.matmul(
                        sc[:, 512 * h + s : 512 * h + 512],
                        lhsT=kT[64 * h : 64 * h + 64, kb * 128 : (kb + 1) * 128],
                        rhs=qT[64 * h : 64 * h + 64, qc * 512 + s : (qc + 1) * 512],
                        start=True,
                        stop=True,
                    )
        pt = pat_pool.tile([128, 1024], MMDT, tag="pat", name=f"pat{b}_{qc}_{kb}")
        pats[u] = pt
        # exp (ACT reads PSUM, scale=1/8 folded in); one instruction for
        # off-diagonal units, two for diagonal (skip the masked-out cols)
        if dd <= 0:
            eranges = [(0, 1024)]
        elif os.environ.get("EXP3D", "1") == "1":
            # single 3D-AP call covering both heads' unmasked columns
            pt3 = pt.rearrange("p (h q) -> p h q", q=512)
            sc3 = sc.rearrange("p (h q) -> p h q", q=512)
            for _d in range(dup_exp):
                nc.scalar.activation(
                    pt3[:, :, s:512],
                    sc3[:, :, s:512],
                    mybir.ActivationFunctionType.Exp,
                    scale=INV_SCALE,
                )
            eranges = []
        else:
            eranges = [(s, 512), (512 + s, 1024)]
        for e0, e1 in eranges:
            for _d in range(dup_exp):
                nc.scalar.activation(
                    pt[:, e0:e1],
                    sc[:, e0:e1],
                    mybir.ActivationFunctionType.Exp,
                    scale=INV_SCALE,
                )


    def emit_pv(u):
        b, qc, kb = u
        last_kb = 4 * qc + 3
        if kb == 0:
            zaccs[(b, qc)] = [
                pz_pool.tile([128, 512], F32, tag="z", name=f"zacc{b}_{qc}_{h}")
                for h in range(HPC)
            ]
        zacc = zaccs[(b, qc)]
        dd = kb - 4 * qc
        s = 128 * dd if dd > 0 else 0
        for h in range(HPC):
            for _d in range(dup_mm):
                nc.tensor.matmul(
                    zacc[h][:, s:512],
                    lhsT=vbigs[b][:, kb * 256 + 128 * h : kb * 256 + 128 * h + 128],
                    rhs=pats[u][:, 512 * h + s : 512 * h + 512],
                    start=(kb == 0 and _d == 0),
                    stop=(kb == last_kb and _d == dup_mm - 1),
                )
        if kb == last_kb:
            emit_normalize(b, qc)

    def emit_normalize(hg, qc):
        # normalize: z = z / denom  (DVE reciprocal + mul, Pool broadcast)
        zacc = zaccs[(hg, qc)]
        zsb = stage_pool.tile([128, 512], MMDT, tag="zsb", name=f"zsb{hg}_{qc}")
        zsbs[(hg, qc)] = zsb
        # denominator rows 64-127 are already replicated (ones*64 in PV
        # weights): reciprocal on [64,512] costs the same as [1,512] on DVE
        # (free-dim serial, partitions parallel) and tensor_mul aligns
        # partition-wise with zacc rows 0-63 -- no Pool broadcast hop.
        rbs = []
        for h in range(HPC):
            r_sb = stage_pool.tile([64, 512], F32, tag="r")
            nc.vector.reciprocal(r_sb, zacc[h][64:128, :])
            rbs.append(r_sb)
        for h in range(HPC):
            for _d in range(dup_dve):
                nc.vector.tensor_mul(
                    zsb[64 * h : 64 * h + 64, :],
                    zacc[h][0:64, :],
                    rbs[h],
                )
        if hg == 1:
            ops = make_proj(qc, zsbs[(0, qc)], zsbs[(1, qc)],
                            tail=(qc == NQC - 1
                                  and os.environ.get("TAIL_ACT", "1") == "1"))
            pending.extend((cur_idx[0] + INJECT_DELAY, op) for op in ops)

    for i, u in enumerate(stream):
        cur_idx[0] = i
        emit_qk_exp(u)
        emit_some_pending(1)
        if i >= PV_LAG:
            emit_pv(stream[i - PV_LAG])
    for u in stream[-PV_LAG:]:
        emit_some_pending(1)
        emit_pv(u)

    # drain whatever projection ops remain at the end of the iteration
    cur_idx[0] = float("inf")
    emit_some_pending(len(pending))


def make_in_maps(q, k, v, W_O):
    import ml_dtypes

    bf16 = ml_dtypes.bfloat16
    q = np.asarray(q, dtype=np.float32).astype(bf16)
    k = np.asarray(k, dtype=np.float32).astype(bf16)
    v = np.asarray(v, dtype=np.float32).astype(bf16)
    W_O = np.asarray(W_O, dtype=np.float32).astype(bf16)
    in_maps = []
    for c in range(N_CORES):
        b = c // 4  # batch owned by this core
        g = c % 4   # head-group of 4 heads
        cols = slice(g * 2 * CW, (g + 1) * 2 * CW)
        in_maps.append(
            {
                "q": np.ascontiguousarray(q[b, :, cols]),
                "k": np.ascontiguousarray(k[b, :, cols]),
                "v": np.ascontiguousarray(v[b, :, cols]),
                "wo": np.ascontiguousarray(
                    W_O[g * 4 : (g + 1) * 4].reshape(2 * CW, D_MODEL)
                ),
            }
        )
    return in_maps


def get_nc():
    if "nc" not in _CACHE:
        _CACHE["nc"] = _build_bass()
    return _CACHE["nc"]


def kernel(q, k, v, residual, W_O, b_O):
    nc = get_nc()
    in_maps = make_in_maps(q, k, v, W_O)
    res = run_bass_kernel_spmd(nc, in_maps, core_ids=list(range(N_CORES)))
    out = np.zeros((B, S, D_MODEL), dtype=np.float64)
    for c, r in enumerate(res.results):
        out[c // 4] += r["out"].astype(np.float64)
    out = (out + np.asarray(b_O, dtype=np.float64)[None, None, :]).astype(np.float32)
    return out, np.asarray(residual)

